# revision 12
# baseline (speedup 1.0000x reference)
"""ContactMapHead Trainium2 kernel (v4: SYRK band, f32r transposes, tuned DMA).

Reference computation (per batch b):
    h = relu(X @ W^T + pb)            # [S, DP]
    scores = (h @ h^T) * cw + cb      # [S, S]  -- symmetric!

Sharding over 8 NeuronCores: core c handles batch b = c//2 with roll
offset off = (c%2)*1024 applied to X on the host. Each core computes
hT = relu(W @ XT + pb) for its full (rolled) batch in float32r, then
emits the circulant band of the symmetric score map: local tile rows
i_t in 0..7 (tiles of 128), local cols j_t in i_t..i_t+8 (9 tiles of
128, never wrapping). Across the two cores of a batch pair plus
host-side transpose mirroring this covers all 16x16 global tiles
exactly (brute-force verified).

float32r (TF32-like) matmuls run 4x faster than fp32 on the PE
(1 cycle/row, vs 4 for fp32 and 1.5 for f32r transposes) at ~1.6e-4
max relative error per 128-deep dot. X/W are rounded to f32r during
the PE transpose, which is where the precision budget is spent anyway.
"""

import numpy as np

from concourse import bacc, masks, mybir, tile

P = 128
B, S, D = 4, 2048, 1024
DP = 256  # projection dim
NCORES = 8
KT = D // P  # 8 k-tiles over D
PT = DP // P  # 2 p-tiles over DP
SBLK = 512
NSB = S // SBLK  # 4 s-blocks
NROW = 8  # local band rows (tiles of 128) per core
BANDW = 9 * P  # 1152 band columns per row
SEG = BANDW // 3  # 384-col band chunks: >=256 keeps f32r at 1 cyc/row

f32 = mybir.dt.float32
f32r = mybir.dt.float32r


def _build_nc():
    nc = bacc.Bacc()
    x = nc.declare_dram_parameter("x", [S, D], f32, isOutput=False)
    w = nc.declare_dram_parameter("w", [DP, D], f32, isOutput=False)
    pb = nc.declare_dram_parameter("pb", [DP], f32, isOutput=False)
    cwb = nc.declare_dram_parameter("cwb", [2], f32, isOutput=False)
    out = nc.declare_dram_parameter("out", [NROW, P, BANDW], f32, isOutput=True)

    with tile.TileContext(nc) as tc:
        _body(nc, tc, x, w, pb, cwb, out)
    nc.compile()
    return nc


def _body(nc, tc, x, w, pb, cwb, out):
    mult = mybir.AluOpType.mult
    add = mybir.AluOpType.add
    Relu = mybir.ActivationFunctionType.Relu
    Ident = mybir.ActivationFunctionType.Identity

    with (
        tc.tile_pool(name="const", bufs=1) as cpool,
        tc.tile_pool(name="xnat", bufs=2) as xpool,
        tc.tile_pool(name="xt", bufs=2) as xtpool,
        tc.tile_pool(name="orow", bufs=3) as opool,
        tc.tile_pool(name="tp", bufs=2, space="PSUM") as tp,
        tc.tile_pool(name="pj", bufs=2, space="PSUM") as pj,
        tc.tile_pool(name="pw", bufs=4, space="PSUM") as pw,
    ):
        # ---- constants ----
        ident = cpool.tile([P, P], f32, tag="ident")
        masks.make_identity(nc, ident[:])
        ident_r = cpool.tile([P, P], f32r, tag="ident_r")
        nc.vector.tensor_copy(ident_r[:], ident[:])

        pb_t = cpool.tile([P, PT], f32, tag="pb_t")
        nc.scalar.dma_start(pb_t[:], pb.ap().rearrange("(t p) -> p t", p=P))

        cwb_t = cpool.tile([P, 2], f32, tag="cwb_t")
        nc.scalar.dma_start(cwb_t[:], cwb.ap().partition_broadcast(P))

        # first x block, in two half-chunks so transposes start early
        xv0 = x.ap()[0:SBLK, :].rearrange("(t p) d -> p t d", p=P)
        xn0 = xpool.tile([P, 4, D], f32r, tag="xn", name="xn0")
        nc.sync.dma_start(xn0[:, 0:2, :], xv0[:, 0:2, :].bitcast(f32r))
        nc.sync.dma_start(xn0[:, 2:4, :], xv0[:, 2:4, :].bitcast(f32r))

        w_nat = cpool.tile([P, PT, D], f32r, tag="w_nat")
        wv = w.ap().rearrange("(t p) d -> p t d", p=P).bitcast(f32r)
        nc.scalar.dma_start(w_nat[:], wv[:])

        # hT for the whole local map; relu writes per (pt, s-block) slices
        ht = cpool.tile([P, PT, S], f32r, tag="ht")

        # PE warm-up: HAM unthrottles after ~3.4us of sustained activity.
        # Burn junk f32r matmuls during the initial x DMA so the real
        # transposes start at 2.4 GHz. Results are never read.
        warm_src = cpool.tile([P, SBLK], f32r, tag="warm_src")
        nc.gpsimd.memset(warm_src[:].bitcast(f32), 0.0)
        for i in range(24):
            wps = tp.tile([P, SBLK], f32, tag="tp", name="warm")
            nc.tensor.matmul(
                wps[:], ident_r[:], warm_src[:], start=True, stop=True
            )

        def transpose_block(xn, k, tps, trange, col0=0):
            for t in trange:
                nc.tensor.transpose(
                    tps[:, t * P - col0 : (t + 1) * P - col0].bitcast(f32r),
                    xn[:, t, k * P : (k + 1) * P],
                    ident_r[:],
                )

        def project(sb, xt):
            for pt in range(PT):
                pjs = pj.tile([P, SBLK], f32, tag="pj", name="pj")
                for k in range(KT):
                    nc.tensor.matmul(
                        pjs[:],
                        wt[:, k, pt * P : (pt + 1) * P],
                        xt[:, k, :],
                        start=(k == 0),
                        stop=(k == KT - 1),
                    )
                nc.scalar.activation(
                    ht[:, pt, sb * SBLK : (sb + 1) * SBLK],
                    pjs[:],
                    Relu,
                    bias=pb_t[:, pt : pt + 1],
                )

        def emit_pair_row(i_t):
            """Band row i_t: out[i_t] = cw * hT_i^T @ hT[band cols] + cb."""
            base = i_t * P
            psums = []
            for pt in range(PT):
                for si in range(3):
                    if pt == 0:
                        psums.append(pw.tile([P, SEG], f32, tag="pw", name="pw"))
                    c0 = base + si * SEG
                    nc.tensor.matmul(
                        psums[si][:],
                        ht[:, pt, base : base + P],
                        ht[:, pt, c0 : c0 + SEG],
                        start=(pt == 0),
                        stop=(pt == PT - 1),
                    )
            orow = opool.tile([P, BANDW], f32, tag="orow", name="orow")
            for si in range(3):
                dst = orow[:, si * SEG : (si + 1) * SEG]
                if (i_t * 3 + si) % 2 == 0:
                    nc.vector.tensor_scalar(
                        dst, psums[si][:], cwb_t[:, 0:1], cwb_t[:, 1:2], mult, add
                    )
                else:
                    nc.scalar.activation(
                        dst, psums[si][:], Ident,
                        bias=cwb_t[:, 1:2], scale=cwb_t[:, 0:1],
                    )
                if i_t >= NROW - 2:
                    nc.sync.dma_start(
                        out.ap()[i_t, :, si * SEG : (si + 1) * SEG], dst
                    )
            if i_t < NROW - 2:
                nc.sync.dma_start(out.ap()[i_t], orow[:])

        # ---- sb0 transposes in two waves (start on the first half-chunk) ----
        xt0 = xtpool.tile([P, KT, SBLK], f32r, tag="xt", name="xt0")
        for half in range(2):
            for k in range(KT):
                tps = tp.tile([P, PT * P], f32, tag="tp", name="tp0")
                transpose_block(xn0, k, tps, range(2 * half, 2 * half + 2),
                                col0=2 * half * P)
                cols = xt0[:, k, 2 * half * P : (2 * half + 2) * P]
                if k % 2 == 0:
                    nc.vector.tensor_copy(cols, tps[:].bitcast(f32r))
                else:
                    nc.scalar.copy(cols, tps[:].bitcast(f32r))

        # WT[k][:, t*P:(t+1)*P] = W[t-tile, k-block]^T
        wt = cpool.tile([P, KT, DP], f32r, tag="wt")
        for k in range(KT):
            tps = tp.tile([P, SBLK], f32, tag="tp", name="tp")
            for t in range(PT):
                nc.tensor.transpose(
                    tps[:, t * P : (t + 1) * P].bitcast(f32r),
                    w_nat[:, t, k * P : (k + 1) * P],
                    ident_r[:],
                )
            nc.vector.tensor_copy(wt[:, k, :], tps[:, 0:DP].bitcast(f32r))

        project(0, xt0)

        # ---- remaining s-blocks ----
        for sb in range(1, NSB):
            xn = xpool.tile([P, 4, D], f32r, tag="xn", name="xn")
            xv = x.ap()[sb * SBLK : (sb + 1) * SBLK, :].rearrange(
                "(t p) d -> p t d", p=P
            ).bitcast(f32r)
            if sb == 1:
                nc.scalar.dma_start(xn[:], xv[:])
            else:
                nc.sync.dma_start(xn[:], xv[:])

            xt = xtpool.tile([P, KT, SBLK], f32r, tag="xt", name="xt")
            for k in range(KT):
                tps = tp.tile([P, SBLK], f32, tag="tp", name="tp")
                transpose_block(xn, k, tps, range(4))
                if k % 2 == 0:
                    nc.vector.tensor_copy(xt[:, k, :], tps[:].bitcast(f32r))
                else:
                    nc.scalar.copy(xt[:, k, :], tps[:].bitcast(f32r))

            project(sb, xt)

            # band rows whose dependencies just completed:
            # row i_t needs hT cols up to i_t*128+1152 -> all of sb<=2 for
            # rows 0..3, sb3 for rows 4..7.
            if sb == 2:
                for i_t in range(4):
                    emit_pair_row(i_t)
            elif sb == 3:
                for i_t in range(4, NROW):
                    emit_pair_row(i_t)


_NC_CACHE = None


def _get_nc():
    global _NC_CACHE
    if _NC_CACHE is None:
        _NC_CACHE = _build_nc()
    return _NC_CACHE


def _make_in_maps(hidden_states, proj_w, proj_b, clf_w, clf_b):
    hs = np.ascontiguousarray(np.asarray(hidden_states, dtype=np.float32))
    wv = np.ascontiguousarray(np.asarray(proj_w, dtype=np.float32))
    pbv = np.ascontiguousarray(np.asarray(proj_b, dtype=np.float32).reshape(DP))
    cwbv = np.array(
        [np.asarray(clf_w).reshape(-1)[0], np.asarray(clf_b).reshape(-1)[0]],
        dtype=np.float32,
    )
    in_maps = []
    for c in range(NCORES):
        b, half = divmod(c, 2)
        xb = hs[b]
        if half:
            xb = np.ascontiguousarray(np.roll(xb, -S // 2, axis=0))
        in_maps.append({"x": xb, "w": wv, "pb": pbv, "cwb": cwbv})
    return in_maps


def _assemble(results):
    scores = np.empty((B, S, S), np.float32)
    for c in range(NCORES):
        b, half = divmod(c, 2)
        o = results[c]["out"]  # [NROW, P, BANDW]
        for i_t in range(NROW):
            gi = i_t + NROW * half
            strip = o[i_t]
            for lj in range(i_t, i_t + 9):
                gj = (lj + NROW * half) % 16
                V = strip[:, (lj - i_t) * P : (lj - i_t + 1) * P]
                scores[b, gi * P : (gi + 1) * P, gj * P : (gj + 1) * P] = V
                if gj != gi:
                    scores[b, gj * P : (gj + 1) * P, gi * P : (gi + 1) * P] = V.T
    return scores


def kernel(hidden_states, proj_w, proj_b, clf_w, clf_b):
    from concourse.bass_utils import run_bass_kernel_spmd

    nc = _get_nc()
    in_maps = _make_in_maps(hidden_states, proj_w, proj_b, clf_w, clf_b)
    res = run_bass_kernel_spmd(nc, in_maps, core_ids=list(range(NCORES)))
    return _assemble(res.results)


def run_traced(hidden_states, proj_w, proj_b, clf_w, clf_b):
    """Like kernel(), but also returns BassKernelResults with trace info."""
    from concourse.bass_utils import run_bass_kernel_spmd

    nc = _get_nc()
    in_maps = _make_in_maps(hidden_states, proj_w, proj_b, clf_w, clf_b)
    res = run_bass_kernel_spmd(
        nc, in_maps, core_ids=list(range(NCORES)), trace=True
    )
    return _assemble(res.results), res


# revision 13
# speedup vs baseline: 1.0003x; 1.0003x over previous
"""ContactMapHead Trainium2 kernel (v4: SYRK band, f32r transposes, tuned DMA).

Reference computation (per batch b):
    h = relu(X @ W^T + pb)            # [S, DP]
    scores = (h @ h^T) * cw + cb      # [S, S]  -- symmetric!

Sharding over 8 NeuronCores: core c handles batch b = c//2 with roll
offset off = (c%2)*1024 applied to X on the host. Each core computes
hT = relu(W @ XT + pb) for its full (rolled) batch in float32r, then
emits the circulant band of the symmetric score map: local tile rows
i_t in 0..7 (tiles of 128), local cols j_t in i_t..i_t+8 (9 tiles of
128, never wrapping). Across the two cores of a batch pair plus
host-side transpose mirroring this covers all 16x16 global tiles
exactly (brute-force verified).

float32r (TF32-like) matmuls run 4x faster than fp32 on the PE
(1 cycle/row, vs 4 for fp32 and 1.5 for f32r transposes) at ~1.6e-4
max relative error per 128-deep dot. X/W are rounded to f32r during
the PE transpose, which is where the precision budget is spent anyway.
"""

import numpy as np

from concourse import bacc, masks, mybir, tile

P = 128
B, S, D = 4, 2048, 1024
DP = 256  # projection dim
NCORES = 8
KT = D // P  # 8 k-tiles over D
PT = DP // P  # 2 p-tiles over DP
SBLK = 512
NSB = S // SBLK  # 4 s-blocks
NROW = 8  # local band rows (tiles of 128) per core
BANDW = 9 * P  # 1152 band columns per row
SEG = BANDW // 3  # 384-col band chunks: >=256 keeps f32r at 1 cyc/row

f32 = mybir.dt.float32
f32r = mybir.dt.float32r


def _build_nc():
    nc = bacc.Bacc()
    x = nc.declare_dram_parameter("x", [S, D], f32, isOutput=False)
    w = nc.declare_dram_parameter("w", [DP, D], f32, isOutput=False)
    pb = nc.declare_dram_parameter("pb", [DP], f32, isOutput=False)
    cwb = nc.declare_dram_parameter("cwb", [2], f32, isOutput=False)
    out = nc.declare_dram_parameter("out", [NROW, P, BANDW], f32, isOutput=True)

    with tile.TileContext(nc) as tc:
        _body(nc, tc, x, w, pb, cwb, out)
    nc.compile()
    return nc


def _body(nc, tc, x, w, pb, cwb, out):
    mult = mybir.AluOpType.mult
    add = mybir.AluOpType.add
    Relu = mybir.ActivationFunctionType.Relu
    Ident = mybir.ActivationFunctionType.Identity

    with (
        tc.tile_pool(name="const", bufs=1) as cpool,
        tc.tile_pool(name="xnat", bufs=2) as xpool,
        tc.tile_pool(name="xt", bufs=2) as xtpool,
        tc.tile_pool(name="orow", bufs=3) as opool,
        tc.tile_pool(name="tp", bufs=2, space="PSUM") as tp,
        tc.tile_pool(name="pj", bufs=2, space="PSUM") as pj,
        tc.tile_pool(name="pw", bufs=4, space="PSUM") as pw,
    ):
        # ---- constants ----
        ident = cpool.tile([P, P], f32, tag="ident")
        masks.make_identity(nc, ident[:])
        ident_r = cpool.tile([P, P], f32r, tag="ident_r")
        nc.vector.tensor_copy(ident_r[:], ident[:])

        pb_t = cpool.tile([P, PT], f32, tag="pb_t")
        nc.scalar.dma_start(pb_t[:], pb.ap().rearrange("(t p) -> p t", p=P))

        cwb_t = cpool.tile([P, 2], f32, tag="cwb_t")
        nc.scalar.dma_start(cwb_t[:], cwb.ap().partition_broadcast(P))

        # first x block, in two half-chunks so transposes start early
        xv0 = x.ap()[0:SBLK, :].rearrange("(t p) d -> p t d", p=P)
        xn0 = xpool.tile([P, 4, D], f32r, tag="xn", name="xn0")
        nc.sync.dma_start(xn0[:, 0:2, :], xv0[:, 0:2, :].bitcast(f32r))
        nc.sync.dma_start(xn0[:, 2:4, :], xv0[:, 2:4, :].bitcast(f32r))

        w_nat = cpool.tile([P, PT, D], f32r, tag="w_nat")
        wv = w.ap().rearrange("(t p) d -> p t d", p=P).bitcast(f32r)
        nc.scalar.dma_start(w_nat[:], wv[:])

        # hT for the whole local map; relu writes per (pt, s-block) slices
        ht = cpool.tile([P, PT, S], f32r, tag="ht")

        # PE warm-up: HAM unthrottles after ~3.4us of sustained activity.
        # Burn junk f32r matmuls during the initial x DMA so the real
        # transposes start at 2.4 GHz. Results are never read.
        warm_src = cpool.tile([P, SBLK], f32r, tag="warm_src")
        nc.gpsimd.memset(warm_src[:].bitcast(f32), 0.0)
        for i in range(12):
            wps = tp.tile([P, SBLK], f32, tag="tp", name="warm")
            nc.tensor.matmul(
                wps[:], ident_r[:], warm_src[:], start=True, stop=True
            )

        def transpose_block(xn, k, tps, trange, col0=0):
            for t in trange:
                nc.tensor.transpose(
                    tps[:, t * P - col0 : (t + 1) * P - col0].bitcast(f32r),
                    xn[:, t, k * P : (k + 1) * P],
                    ident_r[:],
                )

        def project(sb, xt):
            for pt in range(PT):
                pjs = pj.tile([P, SBLK], f32, tag="pj", name="pj")
                for k in range(KT):
                    nc.tensor.matmul(
                        pjs[:],
                        wt[:, k, pt * P : (pt + 1) * P],
                        xt[:, k, :],
                        start=(k == 0),
                        stop=(k == KT - 1),
                    )
                nc.scalar.activation(
                    ht[:, pt, sb * SBLK : (sb + 1) * SBLK],
                    pjs[:],
                    Relu,
                    bias=pb_t[:, pt : pt + 1],
                )

        def emit_pair_row(i_t):
            """Band row i_t: out[i_t] = cw * hT_i^T @ hT[band cols] + cb."""
            base = i_t * P
            psums = []
            for pt in range(PT):
                for si in range(3):
                    if pt == 0:
                        psums.append(pw.tile([P, SEG], f32, tag="pw", name="pw"))
                    c0 = base + si * SEG
                    nc.tensor.matmul(
                        psums[si][:],
                        ht[:, pt, base : base + P],
                        ht[:, pt, c0 : c0 + SEG],
                        start=(pt == 0),
                        stop=(pt == PT - 1),
                    )
            orow = opool.tile([P, BANDW], f32, tag="orow", name="orow")
            for si in range(3):
                dst = orow[:, si * SEG : (si + 1) * SEG]
                if (i_t * 3 + si) % 2 == 0:
                    nc.vector.tensor_scalar(
                        dst, psums[si][:], cwb_t[:, 0:1], cwb_t[:, 1:2], mult, add
                    )
                else:
                    nc.scalar.activation(
                        dst, psums[si][:], Ident,
                        bias=cwb_t[:, 1:2], scale=cwb_t[:, 0:1],
                    )
            nc.sync.dma_start(out.ap()[i_t], orow[:])

        # ---- sb0 transposes in two waves (start on the first half-chunk) ----
        xt0 = xtpool.tile([P, KT, SBLK], f32r, tag="xt", name="xt0")
        for half in range(2):
            for k in range(KT):
                tps = tp.tile([P, PT * P], f32, tag="tp", name="tp0")
                transpose_block(xn0, k, tps, range(2 * half, 2 * half + 2),
                                col0=2 * half * P)
                cols = xt0[:, k, 2 * half * P : (2 * half + 2) * P]
                if k % 2 == 0:
                    nc.vector.tensor_copy(cols, tps[:].bitcast(f32r))
                else:
                    nc.scalar.copy(cols, tps[:].bitcast(f32r))

        # WT[k][:, t*P:(t+1)*P] = W[t-tile, k-block]^T
        wt = cpool.tile([P, KT, DP], f32r, tag="wt")
        for k in range(KT):
            tps = tp.tile([P, SBLK], f32, tag="tp", name="tp")
            for t in range(PT):
                nc.tensor.transpose(
                    tps[:, t * P : (t + 1) * P].bitcast(f32r),
                    w_nat[:, t, k * P : (k + 1) * P],
                    ident_r[:],
                )
            nc.vector.tensor_copy(wt[:, k, :], tps[:, 0:DP].bitcast(f32r))

        project(0, xt0)

        # ---- remaining s-blocks ----
        for sb in range(1, NSB):
            xn = xpool.tile([P, 4, D], f32r, tag="xn", name="xn")
            xv = x.ap()[sb * SBLK : (sb + 1) * SBLK, :].rearrange(
                "(t p) d -> p t d", p=P
            ).bitcast(f32r)
            if sb == 1:
                nc.scalar.dma_start(xn[:], xv[:])
            else:
                nc.sync.dma_start(xn[:], xv[:])

            xt = xtpool.tile([P, KT, SBLK], f32r, tag="xt", name="xt")
            for k in range(KT):
                tps = tp.tile([P, SBLK], f32, tag="tp", name="tp")
                transpose_block(xn, k, tps, range(4))
                if k % 2 == 0:
                    nc.vector.tensor_copy(xt[:, k, :], tps[:].bitcast(f32r))
                else:
                    nc.scalar.copy(xt[:, k, :], tps[:].bitcast(f32r))

            project(sb, xt)

            # band rows whose dependencies just completed:
            # row i_t needs hT cols up to i_t*128+1152 -> all of sb<=2 for
            # rows 0..3, sb3 for rows 4..7.
            if sb == 2:
                for i_t in range(4):
                    emit_pair_row(i_t)
            elif sb == 3:
                for i_t in range(4, NROW):
                    emit_pair_row(i_t)


_NC_CACHE = None


def _get_nc():
    global _NC_CACHE
    if _NC_CACHE is None:
        _NC_CACHE = _build_nc()
    return _NC_CACHE


def _make_in_maps(hidden_states, proj_w, proj_b, clf_w, clf_b):
    hs = np.ascontiguousarray(np.asarray(hidden_states, dtype=np.float32))
    wv = np.ascontiguousarray(np.asarray(proj_w, dtype=np.float32))
    pbv = np.ascontiguousarray(np.asarray(proj_b, dtype=np.float32).reshape(DP))
    cwbv = np.array(
        [np.asarray(clf_w).reshape(-1)[0], np.asarray(clf_b).reshape(-1)[0]],
        dtype=np.float32,
    )
    in_maps = []
    for c in range(NCORES):
        b, half = divmod(c, 2)
        xb = hs[b]
        if half:
            xb = np.ascontiguousarray(np.roll(xb, -S // 2, axis=0))
        in_maps.append({"x": xb, "w": wv, "pb": pbv, "cwb": cwbv})
    return in_maps


def _assemble(results):
    scores = np.empty((B, S, S), np.float32)
    for c in range(NCORES):
        b, half = divmod(c, 2)
        o = results[c]["out"]  # [NROW, P, BANDW]
        for i_t in range(NROW):
            gi = i_t + NROW * half
            strip = o[i_t]
            for lj in range(i_t, i_t + 9):
                gj = (lj + NROW * half) % 16
                V = strip[:, (lj - i_t) * P : (lj - i_t + 1) * P]
                scores[b, gi * P : (gi + 1) * P, gj * P : (gj + 1) * P] = V
                if gj != gi:
                    scores[b, gj * P : (gj + 1) * P, gi * P : (gi + 1) * P] = V.T
    return scores


def kernel(hidden_states, proj_w, proj_b, clf_w, clf_b):
    from concourse.bass_utils import run_bass_kernel_spmd

    nc = _get_nc()
    in_maps = _make_in_maps(hidden_states, proj_w, proj_b, clf_w, clf_b)
    res = run_bass_kernel_spmd(nc, in_maps, core_ids=list(range(NCORES)))
    return _assemble(res.results)


def run_traced(hidden_states, proj_w, proj_b, clf_w, clf_b):
    """Like kernel(), but also returns BassKernelResults with trace info."""
    from concourse.bass_utils import run_bass_kernel_spmd

    nc = _get_nc()
    in_maps = _make_in_maps(hidden_states, proj_w, proj_b, clf_w, clf_b)
    res = run_bass_kernel_spmd(
        nc, in_maps, core_ids=list(range(NCORES)), trace=True
    )
    return _assemble(res.results), res
